# revision 15
# baseline (speedup 1.0000x reference)
"""CTC loss kernel for Trainium2 (8 NeuronCores, data-parallel over batch).

Contract: kernel(**inputs) takes the FULL unsharded inputs
(preds [T,B,C] f32, labels [B,S] int, preds_size [B] int, labels_len [B] int)
and returns the FULL output: scalar f32 loss = sum_b ctc_loss_b / B.

Strategy:
  * Shard batch B=128 across 8 cores (16 samples/core).
  * The memory-bound part is reading preds (434 MB) once for the
    log-softmax denominator Z[t,b] = sum_c exp(preds[t,b,c]).  Streamed
    as 16 contiguous [128, 6632] tiles (8 timesteps x 16 samples per
    tile, C padded 6625->6632); ScalarE does fused exp+accumulate per
    partition, so zp8[(t%8)*16+b, t//8] = Z[t,b] directly.
  * The alpha recursion runs in linear probability space, scale-free
    (Z never divided in; corrected at the end by sum_t active*ln Z),
    using host-precomputed pexp[t,b,s] = exp(preds[t,b,ext[b,s]]).
    Reciprocal renorm (x *= 2^32/rowmax) every 8 steps keeps fp32
    range; ln(rowmax) values are deferred to one batched Ln at the end
    (interleaving Ln with the exp stream would thrash ACT table loads).
  * Freeze at t >= preds_size[b] is exact: active-mask zeroes the shift
    terms and pexp is 1 there, so alpha_new == alpha.
"""

import sys

sys.path.insert(0, "/opt/trn_rl_repo")

import math

import numpy as np

import concourse.bacc as bacc
import concourse.bass as bass
import concourse.mybir as mybir
import concourse.tile as tile
from concourse.bass import _add_dep_helper

F32 = mybir.dt.float32
AF = mybir.ActivationFunctionType
ALU = mybir.AluOpType

# Problem shapes (hardcoded per contract).
T, B, C, S = 128, 128, 6625, 32
L = 2 * S + 1  # 65
NCORES = 8
BL = B // NCORES  # 16
CPAD = 6632  # C padded so rows stay DMA-friendly; pad value exp()s to 0
TG = 8  # timesteps packed per Z tile -> partition p = (t%8)*16 + b
NT = T // TG  # 16 tiles
RENORM_EVERY = 8
RENORM_TS = [t for t in range(1, T) if t % RENORM_EVERY == 0]
NREN = len(RENORM_TS)
POW_OFF = 32  # renorm targets rowmax -> 2^32
PAD_NEG = -1.0e4  # exp() -> 0


def _build_program():
    nc = bacc.Bacc("TRN2", target_bir_lowering=False, debug=False)

    preds_d = nc.dram_tensor("preds", [T, BL, CPAD], F32, kind="ExternalInput")
    # consts packs [pexp (T*L) | skipm (L) | alpha0 (L+2) | actm (T) | selm (L)]
    NCONST = T * L + 3 * L + T + 2
    consts_d = nc.dram_tensor("consts", [BL, NCONST], F32, kind="ExternalInput")
    # aux packs [W8 (fold (t8,b)->b) | actm8 (active mask in zp8 layout)]
    aux_d = nc.dram_tensor("aux", [128, BL + NT], F32, kind="ExternalInput")
    loss_d = nc.dram_tensor("loss", [BL, 1], F32, kind="ExternalOutput")

    with tile.TileContext(nc) as tc:
        with (
            tc.tile_pool(name="const", bufs=1) as const,
            tc.tile_pool(name="pred", bufs=4) as pred,
            tc.tile_pool(name="scratch", bufs=1) as scratch,
            tc.tile_pool(name="psum", bufs=2, space="PSUM") as psum,
            tc.tile_pool(name="small", bufs=2) as small,
        ):
            # issue the first big preds tile before the consts DMA so the
            # Z-stream (the critical path) starts immediately
            ptile0 = pred.tile([128, CPAD], F32, tag="ptile")
            nc.sync.dma_start(
                out=ptile0,
                in_=preds_d[0:TG, :, :].rearrange("t b c -> (t b) c"),
            )
            consts_t = const.tile([BL, NCONST], F32)
            nc.sync.dma_start(out=consts_t, in_=consts_d[:, :])
            o = T * L
            pexp_t = consts_t[:, 0:o]
            skipm_t = consts_t[:, o : o + L]
            alpha0_t = consts_t[:, o + L : o + 2 * L + 2]
            actm_t = consts_t[:, o + 2 * L + 2 : o + 2 * L + 2 + T]
            selm_t = consts_t[:, o + 2 * L + 2 + T : o + 3 * L + 2 + T]
            aux_t = const.tile([128, BL + NT], F32)
            nc.sync.dma_start(out=aux_t, in_=aux_d[:, :])
            w8_t = aux_t[:, 0:BL]
            actm8_t = aux_t[:, BL : BL + NT]

            # Alpha recursion on [BL, 67]: cells at free offsets 2..66,
            # two zero pad cells in front give the s-1 / s-2 shifts as views.
            ab0 = const.tile([BL, L + 2], F32)
            ab1 = const.tile([BL, L + 2], F32)
            # init (t=0) comes fully host-prepped: [0, 0, pexp00, pexp01, 0...]
            nc.vector.tensor_copy(ab0, alpha0_t)
            nc.vector.memset(ab1, 0.0)
            # rowmax values from each renorm, ln'd in one batch at the end
            rbuf = const.tile([BL, max(NREN, 1)], F32)

            # Z accumulators: zp8[(t%8)*16+b, t//8] = Z[t, b]
            zp8 = const.tile([128, NT], F32)

            exp_scr = scratch.tile([128, CPAD], F32)
            last_exp = None
            for k in range(NT):
                if k == 0:
                    ptile = ptile0
                else:
                    ptile = pred.tile([128, CPAD], F32, tag="ptile")
                    # 8 timesteps x 16 samples: one contiguous 3.4MB block
                    nc.sync.dma_start(
                        out=ptile,
                        in_=preds_d[k * TG : (k + 1) * TG, :, :].rearrange(
                            "t b c -> (t b) c"
                        ),
                    )
                last_exp = nc.scalar.activation(
                    exp_scr, ptile, AF.Exp, accum_out=zp8[:, k : k + 1]
                )

            bufs = [ab0, ab1]
            ri = 0
            for t in range(1, T):
                cur = bufs[(t + 1) % 2]
                nxt = bufs[t % 2]
                pexp_v = pexp_t[:, t * L : (t + 1) * L]
                w = small.tile([BL, L], F32, tag="w")
                # w = alpha[s-2]*skip_ok
                nc.vector.tensor_tensor(w, cur[:, 0:L], skipm_t, op=ALU.mult)
                # w += alpha[s-1]
                nc.vector.tensor_tensor(w, w, cur[:, 1 : 1 + L], op=ALU.add)
                # u = w*active_t + alpha[s]   (frozen rows: u = alpha)
                u = small.tile([BL, L], F32, tag="u")
                nc.vector.scalar_tensor_tensor(
                    u, w, actm_t[:, t : t + 1], cur[:, 2 : 2 + L],
                    op0=ALU.mult, op1=ALU.add,
                )
                if t in RENORM_TS:
                    rmax = rbuf[:, ri : ri + 1]
                    ri += 1
                    nc.vector.tensor_reduce(
                        rmax, u, axis=mybir.AxisListType.X, op=ALU.max
                    )
                    rrec = small.tile([BL, 1], F32, tag="rrec")
                    nc.vector.reciprocal(rrec, rmax)
                    rrec2 = small.tile([BL, 1], F32, tag="rrec2")
                    nc.vector.tensor_scalar_mul(rrec2, rrec, float(2.0**POW_OFF))
                    # alpha_nxt = (u * 2^32/rowmax) * pexp_t
                    nc.vector.scalar_tensor_tensor(
                        nxt[:, 2 : 2 + L], u, rrec2, pexp_v,
                        op0=ALU.mult, op1=ALU.mult,
                    )
                else:
                    nc.vector.tensor_tensor(nxt[:, 2 : 2 + L], u, pexp_v, op=ALU.mult)

            final = bufs[(T - 1) % 2]

            # ---- epilogue: all Ln work batched here (one table switch) ----
            # sum_t active*ln Z  from zp8 layout
            lnz8 = small.tile([128, NT], F32, tag="lnz8")
            nc.scalar.activation(lnz8, zp8, AF.Ln)
            lnzm8 = small.tile([128, NT], F32, tag="lnzm8")
            nc.vector.tensor_tensor(lnzm8, lnz8, actm8_t, op=ALU.mult)
            red8 = small.tile([128, 1], F32, tag="red8")
            nc.vector.tensor_reduce(
                red8, lnzm8, axis=mybir.AxisListType.X, op=ALU.add
            )
            slnz = psum.tile([BL, 1], F32, tag="slnz")
            nc.tensor.matmul(slnz, w8_t, red8, start=True, stop=True)

            # lacc = sum of deferred ln(rowmax)
            lnrb = small.tile([BL, NREN], F32, tag="lnrb")
            i_lnrb = nc.scalar.activation(lnrb, rbuf[:, 0:NREN], AF.Ln)
            # keep the ACT queue clear of epilogue Lns until every exp
            # has issued, else the scheduler stalls the Z-stream behind
            # the (recursion-gated) Ln inputs
            _add_dep_helper(i_lnrb.ins, last_exp.ins, sync=False,
                            reason="exps before epilogue lns")
            lacc = small.tile([BL, 1], F32, tag="lacc")
            nc.vector.tensor_reduce(
                lacc, lnrb, axis=mybir.AxisListType.X, op=ALU.add
            )

            # asum = alpha[2*len] + alpha[2*len-1]  (mask-select + row-sum)
            seltmp = small.tile([BL, L], F32, tag="seltmp")
            asum = small.tile([BL, 1], F32, tag="asum")
            nc.vector.tensor_tensor(
                seltmp, final[:, 2 : 2 + L], selm_t, op=ALU.mult
            )
            nc.vector.tensor_reduce(
                asum, seltmp, axis=mybir.AxisListType.X, op=ALU.add
            )
            lnasum = small.tile([BL, 1], F32, tag="lnasum")
            i_lnasum = nc.scalar.activation(lnasum, asum, AF.Ln)
            _add_dep_helper(i_lnasum.ins, last_exp.ins, sync=False,
                            reason="exps before epilogue lns")

            # loss = slnz - lnasum - lacc + NREN*32*ln2
            d1 = small.tile([BL, 1], F32, tag="d1")
            nc.vector.tensor_tensor(d1, slnz, lnasum, op=ALU.subtract)
            d2 = small.tile([BL, 1], F32, tag="d2")
            nc.vector.tensor_tensor(d2, d1, lacc, op=ALU.subtract)
            lossv = small.tile([BL, 1], F32, tag="lossv")
            nc.vector.tensor_scalar_add(
                lossv, d2, float(NREN * POW_OFF * math.log(2.0))
            )
            nc.sync.dma_start(out=loss_d[:, :], in_=lossv)

    nc.finalize()
    return nc


_NC_CACHE = None


def _get_program():
    global _NC_CACHE
    if _NC_CACHE is None:
        _NC_CACHE = _build_program()
    return _NC_CACHE


def _prep_in_maps(preds, labels, preds_size, labels_len):
    preds = np.asarray(preds, dtype=np.float32)
    labels = np.asarray(labels).astype(np.int64)
    preds_size = np.asarray(preds_size).astype(np.int64)
    labels_len = np.asarray(labels_len).astype(np.int64)

    # Extended label sequence: blank, l1, blank, ..., blank  [B, L]
    ext = np.zeros((B, L), dtype=np.int64)
    ext[:, 1::2] = labels
    ext_s2 = np.full((B, L), -1, dtype=np.int64)
    ext_s2[:, 2:] = ext[:, :-2]
    skipm = ((ext != 0) & (ext != ext_s2)).astype(np.float32)

    tgrid = np.arange(T)
    actm = (tgrid[None, :] < preds_size[:, None]).astype(np.float32)

    selm = np.zeros((B, L), dtype=np.float32)
    idx_last = 2 * labels_len
    idx_prev = np.maximum(idx_last - 1, 0)
    np.add.at(selm, (np.arange(B), idx_last), 1.0)
    np.add.at(selm, (np.arange(B), idx_prev), 1.0)

    # pexp[t,b,s] = exp(preds[t,b,ext[b,s]]); 1.0 where t >= preds_size[b]
    gath = np.take_along_axis(
        preds, np.broadcast_to(ext[None, :, :], (T, B, L)), axis=2
    )
    pexp = np.exp(gath.astype(np.float64)).astype(np.float32)
    frozen = tgrid[:, None] >= preds_size[None, :]  # [T, B]
    pexp[frozen, :] = 1.0
    pexp_bt = np.ascontiguousarray(pexp.transpose(1, 0, 2)).reshape(B, T * L)

    preds_pad = np.full((T, B, CPAD), PAD_NEG, dtype=np.float32)
    preds_pad[:, :, :C] = preds

    alpha0 = np.zeros((B, L + 2), dtype=np.float32)
    alpha0[:, 2] = pexp[0, :, 0]
    alpha0[:, 3] = np.where(labels_len > 0, pexp[0, :, 1], 0.0)
    consts_all = np.concatenate([pexp_bt, skipm, alpha0, actm, selm], axis=1)

    # aux (per core): W8 fold matrix + active mask in zp8 layout
    w8 = np.zeros((128, BL), dtype=np.float32)
    w8[np.arange(128), np.arange(128) % BL] = 1.0

    in_maps = []
    for i in range(NCORES):
        sl = slice(i * BL, (i + 1) * BL)
        actm_core = actm[sl]  # [BL, T]
        actm8 = np.zeros((128, NT), dtype=np.float32)
        for p in range(128):
            t8, b = p // BL, p % BL
            actm8[p, :] = actm_core[b, t8::TG]
        aux = np.concatenate([w8, actm8], axis=1)
        in_maps.append(
            {
                "preds": np.ascontiguousarray(preds_pad[:, sl, :]),
                "consts": np.ascontiguousarray(consts_all[sl]),
                "aux": aux,
            }
        )
    return in_maps


def _run(in_maps, trace=False):
    from concourse.bass_utils import run_bass_kernel_spmd

    nc = _get_program()
    res = run_bass_kernel_spmd(
        nc, in_maps, list(range(NCORES)), trace=trace
    )
    per_sample = np.concatenate(
        [res.results[i]["loss"][:, 0] for i in range(NCORES)]
    )
    total = np.float32(per_sample.astype(np.float64).sum() / B)
    return total, per_sample, res


def kernel(preds, labels, preds_size, labels_len):
    in_maps = _prep_in_maps(preds, labels, preds_size, labels_len)
    total, _, _ = _run(in_maps)
    return total


def _install_ntff_hook():
    """The agent image's antenv lacks axon_hooks; synthesize it so
    run_bass_kernel_spmd(trace=True) can capture NTFF profiles."""
    import types

    import antenv

    if "antenv.axon_hooks" in sys.modules:
        return
    mod = types.ModuleType("antenv.axon_hooks")
    holder = [None]
    mod.set_axon_ntff_profile_hook = lambda h: holder.__setitem__(0, h)
    mod.get_axon_ntff_profile_hook = lambda: holder[0]
    sys.modules["antenv.axon_hooks"] = mod
    antenv.axon_hooks = mod
    from trn_agent_boot.trn_boot import _ntff_profile_via_ctypes

    mod.set_axon_ntff_profile_hook(
        _ntff_profile_via_ctypes("/opt/axon/libaxon_pjrt.so")
    )


def kernel_profiled(preds, labels, preds_size, labels_len):
    """Returns (loss, per_sample, BassKernelResults with exec_time_ns)."""
    _install_ntff_hook()
    from concourse import bass_utils

    bass_utils.upload_artifacts = lambda tmpdir: f"local:{tmpdir}"
    in_maps = _prep_in_maps(preds, labels, preds_size, labels_len)
    return _run(in_maps, trace=True)
